# revision 6
# baseline (speedup 1.0000x reference)
"""ConvEncoder kernel for 8 TRN2 NeuronCores.

Computes: emb = emb_table[x]; windows = im2col(pad(emb), WIN=5);
y = gelu(windows @ W.T + b), for x (16, 2048) int32 ids.

Sharding: data-parallel over batch — 2 batches per core × 8 cores.
Per-core pipeline (all on device):
  1. indirect-DMA gather of emb rows (tokens land one-per-partition)
  2. TensorE 128x128 transposes -> embT (EMB on partitions)
  3. 5 shifted matmuls per 512-token span, accumulated in PSUM
  4. ScalarE exact-GELU(+bias) PSUM->SBUF, DMA out
"""

import numpy as np

import concourse.bass as bass
import concourse.mybir as mybir
from concourse import bacc
from concourse.bass import IndirectOffsetOnAxis
from concourse.bass_utils import run_bass_kernel_spmd
from concourse.masks import make_identity
from concourse.tile import TileContext

B, S, EMB, WIN, OUT, VOCAB = 16, 2048, 128, 5, 128, 50257
NCORES = 8
BPC = B // NCORES              # batches per core
T = BPC * S                    # tokens per core (4096)
NTILE = T // 128               # gather/transpose tiles per core (32)
TPB = S // 128                 # tiles per batch (16)
SPAN = 512                     # tokens per matmul psum tile
NSPAN = T // SPAN              # psum tiles per core (8)
SPB = S // SPAN                # spans per batch (4)
HALO = WIN // 2                # 2
EC = S + 2 * HALO              # embT columns per batch (2052)
NGATHER = 4                    # gather split (columns of 32-wide idx tile)

MM_DT = mybir.dt.float32r     # full-rate fp32 TensorE mode (A/B vs float32)

_cache = {}


def _build():
    nc = bacc.Bacc("TRN2", target_bir_lowering=False, debug=False)
    xi = nc.declare_dram_parameter("xi", [128, NTILE], mybir.dt.int32, isOutput=False)
    tbl = nc.declare_dram_parameter("tbl", [VOCAB, EMB], mybir.dt.float32, isOutput=False)
    wt = nc.declare_dram_parameter("wt", [128, WIN * OUT], mybir.dt.float32, isOutput=False)
    bv = nc.declare_dram_parameter("bias", [128, 1], mybir.dt.float32, isOutput=False)
    out = nc.declare_dram_parameter("out", [128, T], mybir.dt.float32, isOutput=True)

    with TileContext(nc) as tc:
        with (
            tc.tile_pool(name="const", bufs=1) as constp,
            tc.tile_pool(name="work", bufs=3) as workp,
            tc.tile_pool(name="tpsum", bufs=4, space="PSUM") as tpsump,
            tc.tile_pool(name="mpsum", bufs=3, space="PSUM") as mpsump,
        ):
            idx_sb = constp.tile([128, NTILE], mybir.dt.int32)
            nc.sync.dma_start(out=idx_sb[:], in_=xi[:])
            wt_f32 = constp.tile([128, WIN * OUT], mybir.dt.float32)
            nc.sync.dma_start(out=wt_f32[:], in_=wt[:])
            wt_sb = constp.tile([128, WIN * OUT], MM_DT)
            nc.vector.tensor_copy(out=wt_sb[:], in_=wt_f32[:])
            b_sb = constp.tile([128, 1], mybir.dt.float32)
            nc.sync.dma_start(out=b_sb[:], in_=bv[:])
            ident = constp.tile([128, 128], mybir.dt.float32)
            make_identity(nc, ident[:])

            # gather buffer: gb[p, c, :] = tbl[idx[p, c], :]
            gb = constp.tile([128, NTILE, EMB], mybir.dt.float32)
            # emb transposed: embT[e, bb*EC + 2 + t] = emb[bb, t, e]; halo cols zero
            embT = constp.tile([128, BPC * EC], MM_DT)
            for bb in range(BPC):
                nc.vector.memset(embT[:, bb * EC : bb * EC + HALO].bitcast(mybir.dt.float32), 0.0)
                nc.vector.memset(embT[:, bb * EC + HALO + S : (bb + 1) * EC].bitcast(mybir.dt.float32), 0.0)

            # HW DGE gathers need a dedicated contiguous [128, 1] offset tile
            for c in range(NTILE):
                idxc = constp.tile([128, 1], mybir.dt.int32, name=f"idxc{c}", tag="idxc", bufs=NTILE)
                nc.vector.tensor_copy(out=idxc[:], in_=idx_sb[:, c : c + 1])
                nc.gpsimd.indirect_dma_start(
                    out=gb[:, c, :],
                    out_offset=None,
                    in_=tbl[:],
                    in_offset=IndirectOffsetOnAxis(ap=idxc[:], axis=0),
                )

            for c in range(NTILE):
                pt = tpsump.tile([128, 128], mybir.dt.float32, space="PSUM", name=f"pt{c}", tag="pt")
                nc.tensor.transpose(out=pt[:], in_=gb[:, c, :], identity=ident[:])
                bb, tl = c // TPB, (c % TPB) * 128
                nc.vector.tensor_copy(
                    out=embT[:, bb * EC + HALO + tl : bb * EC + HALO + tl + 128],
                    in_=pt[:],
                )

            for j in range(NSPAN):
                bb, ts0 = j // SPB, (j % SPB) * SPAN
                ps = mpsump.tile([128, SPAN], mybir.dt.float32, space="PSUM", name=f"ps{j}", tag="ps")
                for k in range(WIN):
                    nc.tensor.matmul(
                        out=ps[:],
                        lhsT=wt_sb[:, k * OUT : (k + 1) * OUT],
                        rhs=embT[:, bb * EC + ts0 + k : bb * EC + ts0 + k + SPAN],
                        start=(k == 0),
                        stop=(k == WIN - 1),
                    )
                ao = workp.tile([128, SPAN], mybir.dt.float32, name=f"ao{j}", tag="ao")
                nc.scalar.activation(
                    out=ao[:], in_=ps[:],
                    func=mybir.ActivationFunctionType.Gelu,
                    bias=b_sb[:, 0:1],
                )
                nc.sync.dma_start(out=out[:, j * SPAN : (j + 1) * SPAN], in_=ao[:])

    nc.compile()
    return nc


def _prep_inputs(x, emb_table, W, b):
    x = np.asarray(x).astype(np.int32)
    emb_table = np.ascontiguousarray(np.asarray(emb_table, dtype=np.float32))
    W = np.asarray(W, dtype=np.float32)
    b = np.asarray(b, dtype=np.float32)
    wt = np.ascontiguousarray(
        W.reshape(OUT, WIN, EMB).transpose(2, 1, 0).reshape(EMB, WIN * OUT)
    )
    bias = np.ascontiguousarray(b.reshape(128, 1))
    in_maps = []
    for core in range(NCORES):
        flat = x[core * BPC : (core + 1) * BPC].reshape(-1)
        xi = np.ascontiguousarray(flat.reshape(NTILE, 128).T)
        in_maps.append({"xi": xi, "tbl": emb_table, "wt": wt, "bias": bias})
    return in_maps


def kernel(x, emb_table, W, b, _trace=False):
    if "nc" not in _cache:
        _cache["nc"] = _build()
    nc = _cache["nc"]
    in_maps = _prep_inputs(x, emb_table, W, b)
    res = run_bass_kernel_spmd(nc, in_maps, core_ids=list(range(NCORES)), trace=_trace)
    _cache["last_result"] = res
    outs = []
    for core in range(NCORES):
        oc = res.results[core]["out"]          # (128, T)
        outs.append(oc.T.reshape(BPC, S, OUT))
    return np.concatenate(outs, axis=0)
